# revision 23
# baseline (speedup 1.0000x reference)
"""Trainium2 Bass kernel for MTGNN temporal classifier (single layer).

Self-contained: takes FULL inputs as numpy arrays, shards across 8 NeuronCores
(batch x time-half), runs one SPMD Bass/Tile program, gathers the full output.

Sharding: core = 2*b + th  (b in 0..3 batches, th in 0..1 time-halves).
The mixprop hop GEMMs (dominant cost) run row-sharded with the normalized
adjacency replicated in SBUF, so the hop chains need no cross-core traffic.
LayerNorm is folded analytically into the skipE convolution, so the only
collective is one pairwise AllGather of [skip01 | rawE | stats] partials.
"""

import numpy as np
import ml_dtypes

import concourse.bass as bass
import concourse.tile as tile
import concourse.bass_isa as bass_isa
from concourse import bacc, mybir
from concourse import bass_utils

BF16 = mybir.dt.bfloat16
F32 = mybir.dt.float32
bf16 = ml_dtypes.bfloat16
AF = mybir.ActivationFunctionType
ALU = mybir.AluOpType

# problem dims
B, C_IN, N, T = 4, 129, 1259, 25
RC, CC, SC, EC, OUT = 128, 126, 128, 128, 64
K = 3
T1 = T - (K - 1)          # 23
NP = 1280                 # padded node count
NV = NP // 128            # 10 node blocks
TAU = 12                  # local output time steps per core (incl. 1 pad on th=1)
TLOC = TAU + 2            # 14 local input time steps
VCH = [(0, 512), (512, 512), (1024, 256)]   # v chunks (full NP)
CNT = float(RC * N * T1)  # layernorm element count per batch
EPS = 1e-5
NQ = 3                    # row-chunk count (4 tau each)

_CACHE = {}


def _build_program(debug_taps=False):
    nc = bacc.Bacc("TRN2", target_bir_lowering=False, debug=False, num_devices=8)

    def din(name, shape, dt=BF16):
        return nc.dram_tensor(name, shape, dt, kind="ExternalInput").ap()

    x_hi = din("x_hi", [128, TLOC, NP])
    x_lo = din("x_lo", [TLOC, NP])          # channel 128, [t, v]
    g1 = din("g1", [128, NV, NP])           # 0.5 * norm_adj(adj).T     padded
    g2 = din("g2", [128, NV, NP])           # 0.5 * norm_adj(adj.T).T   padded
    wsT_hi = din("wsT_hi", [128, 128])
    wsT_lo = din("wsT_lo", [1, 128])
    w0T_hi = din("w0T_hi", [128, TLOC, 128])
    w0T_lo = din("w0T_lo", [1, TLOC, 128])
    wfT = din("wfT", [128, K, CC])
    wgT = din("wgT", [128, K, CC])
    bf_v = din("bf_v", [CC, 1], F32)
    bg_v = din("bg_v", [CC, 1], F32)
    w1T = din("w1T", [CC, TAU, 128])
    wmp1T = din("wmp1T", [128, 4, 128])
    wmp2T = din("wmp2T", [128, 4, 128])
    b_resid_v = din("b_resid_v", [128, 1], F32)
    wET = din("wET", [128, TAU, 128])
    wEsum_v = din("wEsum_v", [128, 1], F32)
    b01_v = din("b01_v", [128, 1], F32)
    we1T = din("we1T", [128, 128])
    be1_v = din("be1_v", [128, 1], F32)
    we2T = din("we2T", [128, OUT])
    be2_v = din("be2_v", [OUT, 1], F32)
    whT = din("whT", [OUT, 1])
    bh_v = din("bh_v", [1, 1], F32)
    tmask = din("tmask", [128, TAU], F32)
    y = nc.dram_tensor("y", [1, NP], F32, kind="ExternalOutput").ap()
    taps = {}
    if debug_taps:
        for nm, shp, dt in [("d_H0", [128, TLOC, NP], BF16),
                            ("d_hcm", [128, TAU, NP], BF16),
                            ("d_xT", [128, 4, NV, 128], BF16),
                            ("d_h1", [128, 4, NV, 128], BF16),
                            ("d_h1cm", [128, 4, NV, 128], BF16),
                            ("d_h2", [128, 4, NV, 128], BF16),
                            ("d_h3", [128, 4, NV, 128], BF16),
                            ("d_h3cm", [128, 4, NV, 128], BF16),
                            ("d_maccmp1", [128, TAU, NP], BF16),
                            ("d_macc", [128, TAU, NP], BF16),
                            ("d_skip", [128, NP], F32),
                            ("d_rawE", [128, NP], F32),
                            ("d_stats", [128, 2], F32)]:
            taps[nm] = nc.dram_tensor(nm, shp, dt, kind="ExternalOutput").ap()

    with tile.TileContext(nc) as tc:
        with (
            tc.tile_pool(name="persist", bufs=1) as pp,
            tc.tile_pool(name="dram", bufs=1, space="DRAM") as dram,
        ):
            # ---- persistent tiles ----
            hcm = pp.tile([128, TAU, NP], BF16)       # f*g, channel-major (rows 126,127 zero)
            macc = pp.tile([128, TAU, NP], BF16)      # residual + mixprop accumulator
            skip_acc = pp.tile([128, NP], F32)        # skip0+skip1 partial
            rawE_acc = pp.tile([128, NP], F32)        # skipE on un-normalized h, partial
            wmp1_t = pp.tile([128, 4, 128], BF16)
            wmp2_t = pp.tile([128, 4, 128], BF16)
            w1T_t = pp.tile([CC, TAU, 128], BF16)
            wET_t = pp.tile([128, TAU, 128], BF16)
            brv = pp.tile([128, 1], F32)
            wEs_t = pp.tile([128, 1], F32)
            b01_t = pp.tile([128, 1], F32)
            we1_t = pp.tile([128, 128], BF16)
            be1_t = pp.tile([128, 1], F32)
            we2_t = pp.tile([128, OUT], BF16)
            be2_t = pp.tile([OUT, 1], F32)
            whT_t = pp.tile([OUT, 1], BF16)
            bh_t = pp.tile([1, 1], F32)
            tmask_t = pp.tile([128, TAU], F32)
            g_first = pp.tile([128, NV, NP], BF16, tag="G", name="g_slot")

            sums_t = pp.tile([128, TAU], F32)
            sqs_t = pp.tile([128, TAU], F32)
            nc.vector.memset(hcm[:], 0.0)
            nc.vector.memset(macc[:, :, N:NP], 0.0)
            nc.vector.memset(rawE_acc[:], 0.0)

            # ================= stage A =================
            with tc.tile_pool(name="stageA", bufs=1) as pa, \
                 tc.tile_pool(name="stag", bufs=2) as pstag:
                xh = pa.tile([128, TLOC, NP], BF16)
                H0 = pa.tile([128, TLOC, NP], BF16)
                ws_hi_t = pa.tile([128, 128], BF16)
                ws_lo_t = pa.tile([1, 128], BF16)
                w0_hi_t = pa.tile([128, TLOC, 128], BF16)
                w0_lo_t = pa.tile([1, TLOC, 128], BF16)
                wf_t = pa.tile([128, K, CC], BF16)
                wg_t = pa.tile([128, K, CC], BF16)
                bfv_t = pa.tile([CC, 1], F32)
                bgv_t = pa.tile([CC, 1], F32)
                for t_, d_ in [(ws_hi_t, wsT_hi), (ws_lo_t, wsT_lo),
                               (w0_hi_t, w0T_hi), (w0_lo_t, w0T_lo)]:
                    nc.scalar.dma_start(t_[:], d_[:])
                for tp_ in range(TLOC):
                    nc.scalar.dma_start(xh[:, tp_, :], x_hi[:, tp_, :])
                for t_, d_ in [(wf_t, wfT), (wg_t, wgT), (bfv_t, bf_v),
                               (bgv_t, bg_v)]:
                    nc.scalar.dma_start(t_[:], d_[:])

                for t_, d_ in [(wmp1_t, wmp1T), (wmp2_t, wmp2T), (w1T_t, w1T),
                               (wET_t, wET), (brv, b_resid_v), (wEs_t, wEsum_v),
                               (b01_t, b01_v), (we1_t, we1T), (be1_t, be1_v),
                               (we2_t, we2T), (be2_t, be2_v), (whT_t, whT),
                               (bh_t, bh_v), (tmask_t, tmask)]:
                    nc.gpsimd.dma_start(t_[:], d_[:])

                # start conv (H0) + skip0, looped over t'
                with tc.tile_pool(name="psA1", bufs=2, space="PSUM") as psA1, \
                     tc.tile_pool(name="psA1s", bufs=1, space="PSUM") as psA1s:
                    s0ps = psA1s.tile([128, 3, 512], F32, tag="s0ps")
                    for tp_ in range(TLOC):
                        stg = pstag.tile([1, NP], BF16, tag="xlo_stage")
                        nc.scalar.dma_start(stg[:], x_lo[tp_:tp_ + 1, :])
                        for i, (vo, vl) in enumerate(VCH):
                            psum = psA1.tile([128, 512], F32, tag="ps_start")
                            nc.tensor.matmul(psum[:, 0:vl], ws_hi_t[:],
                                             xh[:, tp_, vo:vo + vl],
                                             start=True, stop=False)
                            nc.tensor.matmul(psum[:, 0:vl], ws_lo_t[:],
                                             stg[:, vo:vo + vl],
                                             start=False, stop=True)
                            nc.vector.tensor_copy(H0[:, tp_, vo:vo + vl],
                                                  psum[:, 0:vl])
                            nc.tensor.matmul(s0ps[:, i, 0:vl], w0_hi_t[:, tp_, :],
                                             xh[:, tp_, vo:vo + vl],
                                             start=(tp_ == 0), stop=False)
                            nc.tensor.matmul(s0ps[:, i, 0:vl], w0_lo_t[:, tp_, :],
                                             stg[:, vo:vo + vl],
                                             start=False, stop=(tp_ == TLOC - 1))
                    for i, (vo, vl) in enumerate(VCH):
                        nc.vector.tensor_copy(skip_acc[:, vo:vo + vl],
                                              s0ps[:, i, 0:vl])

                # filt/gate inception convs -> hcm = tanh(.) * sigmoid(.)
                # one wide psum + one ACT op per (tau, branch)
                with tc.tile_pool(name="psA2", bufs=1, space="PSUM") as psA2, \
                     tc.tile_pool(name="psA2s", bufs=2, space="PSUM") as psA2s:
                    for tau in range(TAU):
                        psf = psA2.tile([CC, 1536], F32, tag="ps_f")
                        psg = psA2.tile([CC, 1536], F32, tag="ps_g")
                        for vo, vl in VCH:
                            for k in range(K):
                                nc.tensor.matmul(psf[:, vo:vo + vl], wf_t[:, k, :],
                                                 H0[:, tau + k, vo:vo + vl],
                                                 start=(k == 0), stop=(k == K - 1))
                            for k in range(K):
                                nc.tensor.matmul(psg[:, vo:vo + vl], wg_t[:, k, :],
                                                 H0[:, tau + k, vo:vo + vl],
                                                 start=(k == 0), stop=(k == K - 1))
                        fs = pstag.tile([CC, NP], BF16, tag="fs")
                        gs = pstag.tile([CC, NP], BF16, tag="gs")
                        nc.scalar.activation(fs[:], psf[:, 0:NP], AF.Tanh,
                                             bias=bfv_t[:], scale=1.0)
                        nc.scalar.activation(gs[:], psg[:, 0:NP], AF.Sigmoid,
                                             bias=bgv_t[:], scale=1.0)
                        nc.vector.tensor_tensor(hcm[0:CC, tau, :], fs[:], gs[:],
                                                op=ALU.mult)

                    # residual (+ start/mixprop biases) into macc, real nodes
                    nc.vector.tensor_scalar_add(macc[:, :, 0:N],
                                                H0[:, 2:TLOC, 0:N], brv[:])
                    if debug_taps:
                        nc.gpsimd.dma_start(taps["d_H0"][:], H0[:])

                    # skip1 conv partial (contract c,tau over local range)
                    for vo, vl in VCH:
                        psum = psA2s.tile([128, 512], F32, tag="ps_s1")
                        for tau in range(TAU):
                            nc.tensor.matmul(psum[:, 0:vl], w1T_t[:, tau, :],
                                             hcm[0:CC, tau, vo:vo + vl],
                                             start=(tau == 0), stop=(tau == TAU - 1))
                        nc.vector.tensor_tensor(skip_acc[:, vo:vo + vl],
                                                skip_acc[:, vo:vo + vl],
                                                psum[:, 0:vl], op=ALU.add)

            # early pairwise AllGather of skip01 partials (overlaps mixprop)
            cc1_in = dram.tile([128, NP], F32)
            cc1_out = dram.tile([256, NP], F32)
            nc.gpsimd.dma_start(cc1_in[:], skip_acc[:])
            nc.gpsimd.collective_compute(
                "AllGather", ALU.bypass,
                ins=[cc1_in.opt()], outs=[cc1_out.opt()],
                replica_groups=[[0, 1], [2, 3], [4, 5], [6, 7]])

            # AG1 result: combine halves in place via accumulate-DMA
            # (skip_acc's own value was already copied into cc1_in)
            nc.gpsimd.dma_start(skip_acc[:], cc1_out[0:128, :])
            nc.gpsimd.dma_start(skip_acc[:], cc1_out[128:256, :],
                                accum_op=ALU.add)

            # ================= mixprop =================
            with tc.tile_pool(name="mx", bufs=1) as mx, \
                 tc.tile_pool(name="mxT", bufs=2) as mxT, \
                 tc.tile_pool(name="psM", bufs=3, space="PSUM") as ps, \
                 tc.tile_pool(name="psM2", bufs=2, space="PSUM") as ps2:
                for mp in range(2):
                    g = g_first if mp == 0 else pp.tile([128, NV, NP], BF16,
                                                        tag="G", name="g_slot2")
                    nc.gpsimd.dma_start(g[:], (g1 if mp == 0 else g2)[:])
                    wmp = wmp1_t if mp == 0 else wmp2_t
                    for q in range(NQ):
                        # node-major x^T for this row chunk, layout [vl, tau, vb, c]
                        # (xbar j-major fold: transposing hcm[:,t,:] [128c,1280v]
                        #  gives out[p=vl, j=vb, f=c])
                        xT = mxT.tile([128, 4, NV, 128], BF16, tag="xT")
                        for ti in range(4):
                            nc.sync.dma_start_transpose(xT[:, ti, :, :],
                                                        hcm[:, 4 * q + ti, :])
                        if debug_taps and mp == 0 and q == 0:
                            nc.gpsimd.dma_start(taps["d_xT"][:], xT[:])
                        hk_cm = []
                        cur = xT
                        for k in range(3):
                            nxt = mx.tile([128, 4, NV, 128], BF16,
                                          tag=f"hT{k % 2}", name=f"hT{k % 2}")
                            for vb in range(NV):
                                psum = ps.tile([128, 512], F32, tag="ps_hop")
                                for wb in range(NV):
                                    nc.tensor.matmul(
                                        psum[:],
                                        g[:, wb, 128 * vb:128 * (vb + 1)],
                                        cur[:, :, wb, :],
                                        start=(wb == 0), stop=(wb == NV - 1))
                                nc.vector.scalar_tensor_tensor(
                                    nxt[:, :, vb, :], xT[:, :, vb, :], 0.5,
                                    psum[:], ALU.mult, ALU.add)
                            # back to channel-major [c, tau, vb, vl]
                            hk = mx.tile([128, 4, NV, 128], BF16,
                                         tag=f"hkcm{k}", name=f"hkcm{k}")
                            for ti in range(4):
                                nc.sync.dma_start_transpose(hk[:, ti, :, :],
                                                            nxt[:, ti, :, :])
                            if debug_taps and mp == 0 and q == 0:
                                if k == 0:
                                    nc.gpsimd.dma_start(taps["d_h1"][:], nxt[:])
                                    nc.gpsimd.dma_start(taps["d_h1cm"][:], hk[:])
                                elif k == 1:
                                    nc.gpsimd.dma_start(taps["d_h2"][:], nxt[:])
                                else:
                                    nc.gpsimd.dma_start(taps["d_h3"][:], nxt[:])
                                    nc.gpsimd.dma_start(taps["d_h3cm"][:], hk[:])
                            hk_cm.append(hk)
                            cur = nxt
                        # conv1x1 over [x, h1, h2, h3] -> accumulate into macc
                        for ti in range(4):
                            for ci, (vo, vl) in enumerate(VCH):
                                vb0, nvb = vo // 128, vl // 128
                                psum = ps2.tile([128, vl], F32, tag="ps_mpc")
                                nc.tensor.matmul(psum[:], wmp[:, 0, :],
                                                 hcm[:, 4 * q + ti, vo:vo + vl],
                                                 start=True, stop=False)
                                for k in range(3):
                                    nc.tensor.matmul(
                                        psum[:], wmp[:, k + 1, :],
                                        hk_cm[k][:, ti, vb0:vb0 + nvb, :],
                                        start=False, stop=(k == 2))
                                hi = min(vo + vl, N)
                                nc.vector.tensor_tensor(
                                    macc[:, 4 * q + ti, vo:hi],
                                    macc[:, 4 * q + ti, vo:hi],
                                    psum[:, 0:hi - vo], op=ALU.add)
                        if mp == 1:
                            # macc rows 4q..4q+4 final: stats + rawE now
                            for ti in range(4):
                                tau = 4 * q + ti
                                nc.vector.reduce_sum(sums_t[:, tau:tau + 1],
                                                     macc[:, tau, :],
                                                     axis=mybir.AxisListType.X)
                                scr = mxT.tile([128, NP], BF16, tag="sq_scr")
                                nc.scalar.activation(scr[:], macc[:, tau, :],
                                                     AF.Square,
                                                     accum_out=sqs_t[:, tau:tau + 1])
                            for vo, vl in VCH:
                                psum = ps2.tile([128, 512], F32, tag="ps_rEq")
                                for ti in range(4):
                                    nc.tensor.matmul(
                                        psum[:, 0:vl], wET_t[:, 4 * q + ti, :],
                                        macc[:, 4 * q + ti, vo:vo + vl],
                                        start=(ti == 0), stop=(ti == 3))
                                nc.vector.tensor_tensor(rawE_acc[:, vo:vo + vl],
                                                        rawE_acc[:, vo:vo + vl],
                                                        psum[:, 0:vl], op=ALU.add)
                    if debug_taps and mp == 0:
                        nc.gpsimd.dma_start(taps["d_maccmp1"][:], macc[:])

            if debug_taps:
                nc.gpsimd.dma_start(taps["d_hcm"][:], hcm[:])
                nc.gpsimd.dma_start(taps["d_macc"][:], macc[:])
                nc.gpsimd.dma_start(taps["d_skip"][:], skip_acc[:])
            # ================= rawE + stats + collective =================
            with tc.tile_pool(name="late", bufs=1) as pl, \
                 tc.tile_pool(name="psL", bufs=1, space="PSUM") as ps:
                if debug_taps:
                    nc.gpsimd.dma_start(taps["d_rawE"][:], rawE_acc[:])

                stats_p = pl.tile([128, 2], F32)
                msum = pl.tile([128, TAU], F32)
                nc.vector.tensor_tensor(msum[:], sums_t[:], tmask_t[:], op=ALU.mult)
                nc.vector.reduce_sum(stats_p[:, 0:1], msum[:],
                                     axis=mybir.AxisListType.X)
                nc.vector.tensor_tensor(msum[:], sqs_t[:], tmask_t[:], op=ALU.mult)
                nc.vector.reduce_sum(stats_p[:, 1:2], msum[:],
                                     axis=mybir.AxisListType.X)
                if debug_taps:
                    nc.gpsimd.dma_start(taps["d_stats"][:], stats_p[:])

                cc2_in = dram.tile([128, NP + 2], F32)
                cc2_out = dram.tile([256, NP + 2], F32)
                nc.gpsimd.dma_start(cc2_in[:, 0:NP], rawE_acc[:])
                nc.gpsimd.dma_start(cc2_in[:, NP:NP + 2], stats_p[:])
                nc.gpsimd.collective_compute(
                    "AllGather", ALU.bypass,
                    ins=[cc2_in.opt()], outs=[cc2_out.opt()],
                    replica_groups=[[0, 1], [2, 3], [4, 5], [6, 7]])
                rawE_c = pl.tile([128, NP + 2], F32)
                nc.gpsimd.dma_start(rawE_c[:], cc2_out[0:128, :])
                nc.gpsimd.dma_start(rawE_c[:], cc2_out[128:256, :],
                                    accum_op=ALU.add)

                # layernorm scalars (same value on every partition)
                st_r = pl.tile([128, 2], F32)
                nc.gpsimd.partition_all_reduce(st_r[:], rawE_c[:, NP:NP + 2],
                                               channels=128,
                                               reduce_op=bass_isa.ReduceOp.add)
                mv = pl.tile([128, 1], F32)
                msqv = pl.tile([128, 1], F32)
                varv = pl.tile([128, 1], F32)
                m2v = pl.tile([128, 1], F32)
                svv = pl.tile([128, 1], F32)
                rv = pl.tile([128, 1], F32)
                rmv = pl.tile([128, 1], F32)
                bias_c = pl.tile([128, 1], F32)
                nc.vector.tensor_scalar_mul(mv[:], st_r[:, 0:1], 1.0 / CNT)
                nc.vector.tensor_scalar_mul(msqv[:], st_r[:, 1:2], 1.0 / CNT)
                nc.vector.tensor_tensor(m2v[:], mv[:], mv[:], op=ALU.mult)
                nc.vector.tensor_scalar(varv[:], msqv[:], m2v[:], EPS,
                                        op0=ALU.subtract, op1=ALU.add)
                nc.scalar.sqrt(svv[:], varv[:])
                nc.vector.reciprocal(rv[:], svv[:])
                nc.vector.tensor_scalar(rmv[:], rv[:], mv[:], -1.0,
                                        op0=ALU.mult, op1=ALU.mult)
                # bias_c = b01 - r*m*wEsum
                nc.vector.scalar_tensor_tensor(bias_c[:], wEs_t[:], rmv[:],
                                               b01_t[:], ALU.mult, ALU.add)
                # skip_pre = skip01 + r*rawE ; relu with bias
                skip_pre = pl.tile([128, NP], F32)
                nc.vector.scalar_tensor_tensor(skip_pre[:], rawE_c[:, 0:NP],
                                               rv[:], skip_acc[:],
                                               ALU.mult, ALU.add)
                rsk = pl.tile([128, NP], BF16)
                nc.scalar.activation(rsk[:], skip_pre[:], AF.Relu,
                                     bias=bias_c[:], scale=1.0)

                # end convs + head (wide psum, one ACT per stage)
                o1 = pl.tile([128, NP], BF16)
                o2 = pl.tile([OUT, NP], BF16)
                y_sb = pl.tile([1, NP], F32)
                ps1 = ps.tile([128, 1536], F32, tag="ps_e1")
                for vo, vl in VCH:
                    nc.tensor.matmul(ps1[:, vo:vo + vl], we1_t[:],
                                     rsk[:, vo:vo + vl], start=True, stop=True)
                nc.scalar.activation(o1[:], ps1[:, 0:NP], AF.Relu,
                                     bias=be1_t[:], scale=1.0)
                ps2 = ps.tile([OUT, 1536], F32, tag="ps_e2")
                for vo, vl in VCH:
                    nc.tensor.matmul(ps2[:, vo:vo + vl], we2_t[:],
                                     o1[:, vo:vo + vl], start=True, stop=True)
                nc.scalar.activation(o2[:], ps2[:, 0:NP], AF.Identity,
                                     bias=be2_t[:], scale=1.0)
                psh = ps.tile([1, 1536], F32, tag="ps_e1", name="psh")
                for vo, vl in VCH:
                    nc.tensor.matmul(psh[:, vo:vo + vl], whT_t[:],
                                     o2[:, vo:vo + vl], start=True, stop=True)
                nc.scalar.activation(y_sb[:], psh[:, 0:NP], AF.Sigmoid,
                                     bias=bh_t[:], scale=1.0)
                nc.gpsimd.dma_start(y[:], y_sb[:])

    nc.compile()
    return nc


def _norm_adj_T_half(a):
    """0.5 * norm_adj(a).T zero-padded to [NP, NP], bf16."""
    an = a + np.eye(N, dtype=np.float32)
    an = an / an.sum(axis=1, keepdims=True)
    g = 0.5 * an.T
    gp = np.zeros((NP, NP), dtype=np.float32)
    gp[:N, :N] = g
    return gp.reshape(NV, 128, NP).transpose(1, 0, 2).astype(bf16)


def _prep_inputs(inputs):
    x = np.asarray(inputs["x"], np.float32)
    adj = np.asarray(inputs["adj"], np.float32)
    w_start = np.asarray(inputs["w_start"], np.float32)
    b_start = np.asarray(inputs["b_start"], np.float32)
    w_filt = np.asarray(inputs["w_filt"], np.float32)[:, :, 0, :]
    b_filt = np.asarray(inputs["b_filt"], np.float32)
    w_gate = np.asarray(inputs["w_gate"], np.float32)[:, :, 0, :]
    b_gate = np.asarray(inputs["b_gate"], np.float32)
    w_skip0 = np.asarray(inputs["w_skip0"], np.float32)[:, :, 0, :]
    b_skip0 = np.asarray(inputs["b_skip0"], np.float32)
    w_skip1 = np.asarray(inputs["w_skip1"], np.float32)[:, :, 0, :]
    b_skip1 = np.asarray(inputs["b_skip1"], np.float32)
    w_mp1 = np.asarray(inputs["w_mp1"], np.float32)
    b_mp1 = np.asarray(inputs["b_mp1"], np.float32)
    w_mp2 = np.asarray(inputs["w_mp2"], np.float32)
    b_mp2 = np.asarray(inputs["b_mp2"], np.float32)
    w_skipE = np.asarray(inputs["w_skipE"], np.float32)[:, :, 0, :]
    b_skipE = np.asarray(inputs["b_skipE"], np.float32)
    w_end1 = np.asarray(inputs["w_end1"], np.float32)
    b_end1 = np.asarray(inputs["b_end1"], np.float32)
    w_end2 = np.asarray(inputs["w_end2"], np.float32)
    b_end2 = np.asarray(inputs["b_end2"], np.float32)
    w_head = np.asarray(inputs["w_head"], np.float32)
    b_head = np.asarray(inputs["b_head"], np.float32)

    g1 = _norm_adj_T_half(adj)
    g2 = _norm_adj_T_half(adj.T)

    # shared (core-independent) tensors
    wsT = w_start.T  # [129, 128]
    shared = {
        "g1": g1, "g2": g2,
        "wsT_hi": wsT[:128].astype(bf16),
        "wsT_lo": wsT[128:129].astype(bf16),
        "wfT": w_filt.transpose(1, 2, 0).astype(bf16),
        "wgT": w_gate.transpose(1, 2, 0).astype(bf16),
        "bf_v": (b_filt + w_filt.sum(2) @ b_start).reshape(CC, 1).astype(np.float32),
        "bg_v": (b_gate + w_gate.sum(2) @ b_start).reshape(CC, 1).astype(np.float32),
        "b_resid_v": (b_start + b_mp1 + b_mp2).reshape(128, 1).astype(np.float32),
        "wEsum_v": w_skipE.sum((1, 2)).reshape(128, 1).astype(np.float32),
        "b01_v": (b_skip0 + b_skip1 + b_skipE).reshape(128, 1).astype(np.float32),
        "we1T": w_end1.T.astype(bf16),
        "be1_v": b_end1.reshape(128, 1).astype(np.float32),
        "we2T": w_end2.T.astype(bf16),
        "be2_v": b_end2.reshape(OUT, 1).astype(np.float32),
        "whT": w_head.T.astype(bf16),
        "bh_v": b_head.reshape(1, 1).astype(np.float32),
    }
    # w_mp as [c(128 pad), k, o]
    for nm, w in (("wmp1T", w_mp1), ("wmp2T", w_mp2)):
        arr = np.zeros((128, 4, 128), np.float32)
        for k in range(4):
            arr[:CC, k, :] = w[:, k * CC:(k + 1) * CC].T
        shared[nm] = arr.astype(bf16)

    in_maps = []
    for core in range(8):
        b, th = core // 2, core % 2
        t_lo = 0 if th == 0 else TAU
        # x slice [129, 1280, TLOC] zero-padded in nodes and t
        xp = np.zeros((C_IN, TLOC, NP), np.float32)
        t_hi = min(t_lo + TLOC, T)
        xp[:, 0:t_hi - t_lo, :N] = x[b, :, :, t_lo:t_hi].transpose(0, 2, 1)
        # skip0 weight slots aligned to local t: core owns t range
        w0T = np.zeros((C_IN, TLOC, 128), np.float32)
        own_lo, own_hi = (0, 13) if th == 0 else (13, T)
        for tp_ in range(TLOC):
            tg = t_lo + tp_
            if own_lo <= tg < own_hi:
                w0T[:, tp_, :] = w_skip0[:, :, tg].T
        # skip1 / skipE weight slots aligned to local tau
        w1Ta = np.zeros((CC, TAU, 128), np.float32)
        wETa = np.zeros((128, TAU, 128), np.float32)
        for tau in range(TAU):
            tg = t_lo + tau
            if tg < T1:
                w1Ta[:, tau, :] = w_skip1[:, :, tg].T
                wETa[:, tau, :] = w_skipE[:, :, tg].T
        tm = np.ones((128, TAU), np.float32)
        if th == 1:
            tm[:, T1 - TAU:] = 0.0  # tau slots beyond T1 are padding
        m = dict(shared)
        m["x_hi"] = xp[:128].astype(bf16)
        m["x_lo"] = xp[128].astype(bf16)
        m["w0T_hi"] = w0T[:128].astype(bf16)
        m["w0T_lo"] = w0T[128:129].astype(bf16)
        m["w1T"] = w1Ta.astype(bf16)
        m["wET"] = wETa.astype(bf16)
        m["tmask"] = tm
        in_maps.append(m)
    return in_maps


def kernel(**inputs):
    if "nc" not in _CACHE:
        _CACHE["nc"] = _build_program()
    nc = _CACHE["nc"]
    in_maps = _prep_inputs(inputs)
    res = bass_utils.run_bass_kernel_spmd(nc, in_maps, core_ids=list(range(8)))
    out = np.empty((B, N), np.float32)
    for b in range(B):
        out[b] = res.results[2 * b]["y"][0, :N]
    return out


# revision 24
# speedup vs baseline: 1.0250x; 1.0250x over previous
"""Trainium2 Bass kernel for MTGNN temporal classifier (single layer).

Self-contained: takes FULL inputs as numpy arrays, shards across 8 NeuronCores
(batch x time-half), runs one SPMD Bass/Tile program, gathers the full output.

Sharding: core = 2*b + th  (b in 0..3 batches, th in 0..1 time-halves).
The mixprop hop GEMMs (dominant cost) run row-sharded with the normalized
adjacency replicated in SBUF, so the hop chains need no cross-core traffic.
LayerNorm is folded analytically into the skipE convolution, so the only
collective is one pairwise AllGather of [skip01 | rawE | stats] partials.
"""

import numpy as np
import ml_dtypes

import concourse.bass as bass
import concourse.tile as tile
import concourse.bass_isa as bass_isa
from concourse import bacc, mybir
from concourse import bass_utils

BF16 = mybir.dt.bfloat16
F32 = mybir.dt.float32
bf16 = ml_dtypes.bfloat16
AF = mybir.ActivationFunctionType
ALU = mybir.AluOpType

# problem dims
B, C_IN, N, T = 4, 129, 1259, 25
RC, CC, SC, EC, OUT = 128, 126, 128, 128, 64
K = 3
T1 = T - (K - 1)          # 23
NP = 1280                 # padded node count
NV = NP // 128            # 10 node blocks
TAU = 12                  # local output time steps per core (incl. 1 pad on th=1)
TLOC = TAU + 2            # 14 local input time steps
VCH = [(0, 512), (512, 512), (1024, 256)]   # v chunks (full NP)
CNT = float(RC * N * T1)  # layernorm element count per batch
EPS = 1e-5
NQ = 3                    # row-chunk count (4 tau each)

_CACHE = {}


def _build_program(debug_taps=False):
    nc = bacc.Bacc("TRN2", target_bir_lowering=False, debug=False, num_devices=8)

    def din(name, shape, dt=BF16):
        return nc.dram_tensor(name, shape, dt, kind="ExternalInput").ap()

    x_hi = din("x_hi", [128, TLOC, NP])
    x_lo = din("x_lo", [TLOC, NP])          # channel 128, [t, v]
    g1 = din("g1", [128, NV, NP])           # 0.5 * norm_adj(adj).T     padded
    g2 = din("g2", [128, NV, NP])           # 0.5 * norm_adj(adj.T).T   padded
    wsT_hi = din("wsT_hi", [128, 128])
    wsT_lo = din("wsT_lo", [1, 128])
    w0T_hi = din("w0T_hi", [128, TLOC, 128])
    w0T_lo = din("w0T_lo", [1, TLOC, 128])
    wfT = din("wfT", [128, K, CC])
    wgT = din("wgT", [128, K, CC])
    bf_v = din("bf_v", [CC, 1], F32)
    bg_v = din("bg_v", [CC, 1], F32)
    w1T = din("w1T", [CC, TAU, 128])
    wmp1T = din("wmp1T", [128, 4, 128])
    wmp2T = din("wmp2T", [128, 4, 128])
    b_resid_v = din("b_resid_v", [128, 1], F32)
    wET = din("wET", [128, TAU, 128])
    wEsum_v = din("wEsum_v", [128, 1], F32)
    b01_v = din("b01_v", [128, 1], F32)
    we1T = din("we1T", [128, 128])
    be1_v = din("be1_v", [128, 1], F32)
    we2T = din("we2T", [128, OUT])
    be2_v = din("be2_v", [OUT, 1], F32)
    whT = din("whT", [OUT, 1])
    bh_v = din("bh_v", [1, 1], F32)
    tmask = din("tmask", [128, TAU], F32)
    y = nc.dram_tensor("y", [1, NP], F32, kind="ExternalOutput").ap()
    taps = {}
    if debug_taps:
        for nm, shp, dt in [("d_H0", [128, TLOC, NP], BF16),
                            ("d_hcm", [128, TAU, NP], BF16),
                            ("d_xT", [128, 4, NV, 128], BF16),
                            ("d_h1", [128, 4, NV, 128], BF16),
                            ("d_h1cm", [128, 4, NV, 128], BF16),
                            ("d_h2", [128, 4, NV, 128], BF16),
                            ("d_h3", [128, 4, NV, 128], BF16),
                            ("d_h3cm", [128, 4, NV, 128], BF16),
                            ("d_maccmp1", [128, TAU, NP], BF16),
                            ("d_macc", [128, TAU, NP], BF16),
                            ("d_skip", [128, NP], F32),
                            ("d_rawE", [128, NP], F32),
                            ("d_stats", [128, 2], F32)]:
            taps[nm] = nc.dram_tensor(nm, shp, dt, kind="ExternalOutput").ap()

    with tile.TileContext(nc) as tc:
        with (
            tc.tile_pool(name="persist", bufs=1) as pp,
            tc.tile_pool(name="dram", bufs=1, space="DRAM") as dram,
        ):
            # ---- persistent tiles ----
            hcm = pp.tile([128, TAU, NP], BF16)       # f*g, channel-major (rows 126,127 zero)
            macc = pp.tile([128, TAU, NP], BF16)      # residual + mixprop accumulator
            skip_acc = pp.tile([128, NP], F32)        # skip0+skip1 partial
            rawE_acc = pp.tile([128, NP], F32)        # skipE on un-normalized h, partial
            wmp1_t = pp.tile([128, 4, 128], BF16)
            wmp2_t = pp.tile([128, 4, 128], BF16)
            w1T_t = pp.tile([CC, TAU, 128], BF16)
            wET_t = pp.tile([128, TAU, 128], BF16)
            brv = pp.tile([128, 1], F32)
            wEs_t = pp.tile([128, 1], F32)
            b01_t = pp.tile([128, 1], F32)
            we1_t = pp.tile([128, 128], BF16)
            be1_t = pp.tile([128, 1], F32)
            we2_t = pp.tile([128, OUT], BF16)
            be2_t = pp.tile([OUT, 1], F32)
            whT_t = pp.tile([OUT, 1], BF16)
            bh_t = pp.tile([1, 1], F32)
            tmask_t = pp.tile([128, TAU], F32)
            g_first = pp.tile([128, NV, NP], BF16, tag="G", name="g_slot")

            sums_t = pp.tile([128, TAU], F32)
            sqs_t = pp.tile([128, TAU], F32)
            nc.vector.memset(hcm[:], 0.0)
            nc.vector.memset(macc[:, :, N:NP], 0.0)
            nc.vector.memset(rawE_acc[:], 0.0)

            # ================= stage A =================
            with tc.tile_pool(name="stageA", bufs=1) as pa, \
                 tc.tile_pool(name="stag", bufs=2) as pstag:
                xh = pa.tile([128, TLOC, NP], BF16)
                H0 = pa.tile([128, TLOC, NP], BF16)
                ws_hi_t = pa.tile([128, 128], BF16)
                ws_lo_t = pa.tile([1, 128], BF16)
                w0_hi_t = pa.tile([128, TLOC, 128], BF16)
                w0_lo_t = pa.tile([1, TLOC, 128], BF16)
                wf_t = pa.tile([128, K, CC], BF16)
                wg_t = pa.tile([128, K, CC], BF16)
                bfv_t = pa.tile([CC, 1], F32)
                bgv_t = pa.tile([CC, 1], F32)
                for t_, d_ in [(ws_hi_t, wsT_hi), (ws_lo_t, wsT_lo),
                               (w0_hi_t, w0T_hi), (w0_lo_t, w0T_lo)]:
                    nc.sync.dma_start(t_[:], d_[:])
                for tp_ in range(TLOC):
                    nc.scalar.dma_start(xh[:, tp_, :], x_hi[:, tp_, :])
                for t_, d_ in [(wf_t, wfT), (wg_t, wgT), (bfv_t, bf_v),
                               (bgv_t, bg_v)]:
                    nc.sync.dma_start(t_[:], d_[:])

                for t_, d_ in [(wmp1_t, wmp1T), (wmp2_t, wmp2T), (w1T_t, w1T),
                               (wET_t, wET), (brv, b_resid_v), (wEs_t, wEsum_v),
                               (b01_t, b01_v), (we1_t, we1T), (be1_t, be1_v),
                               (we2_t, we2T), (be2_t, be2_v), (whT_t, whT),
                               (bh_t, bh_v), (tmask_t, tmask)]:
                    nc.gpsimd.dma_start(t_[:], d_[:])

                # start conv (H0) + skip0, looped over t'
                with tc.tile_pool(name="psA1", bufs=2, space="PSUM") as psA1, \
                     tc.tile_pool(name="psA1s", bufs=1, space="PSUM") as psA1s:
                    s0ps = psA1s.tile([128, 3, 512], F32, tag="s0ps")
                    for tp_ in range(TLOC):
                        stg = pstag.tile([1, NP], BF16, tag="xlo_stage")
                        nc.sync.dma_start(stg[:], x_lo[tp_:tp_ + 1, :])
                        for i, (vo, vl) in enumerate(VCH):
                            psum = psA1.tile([128, 512], F32, tag="ps_start")
                            nc.tensor.matmul(psum[:, 0:vl], ws_hi_t[:],
                                             xh[:, tp_, vo:vo + vl],
                                             start=True, stop=False)
                            nc.tensor.matmul(psum[:, 0:vl], ws_lo_t[:],
                                             stg[:, vo:vo + vl],
                                             start=False, stop=True)
                            nc.vector.tensor_copy(H0[:, tp_, vo:vo + vl],
                                                  psum[:, 0:vl])
                            nc.tensor.matmul(s0ps[:, i, 0:vl], w0_hi_t[:, tp_, :],
                                             xh[:, tp_, vo:vo + vl],
                                             start=(tp_ == 0), stop=False)
                            nc.tensor.matmul(s0ps[:, i, 0:vl], w0_lo_t[:, tp_, :],
                                             stg[:, vo:vo + vl],
                                             start=False, stop=(tp_ == TLOC - 1))
                    for i, (vo, vl) in enumerate(VCH):
                        nc.vector.tensor_copy(skip_acc[:, vo:vo + vl],
                                              s0ps[:, i, 0:vl])

                # filt/gate inception convs -> hcm = tanh(.) * sigmoid(.)
                # one wide psum + one ACT op per (tau, branch)
                with tc.tile_pool(name="psA2", bufs=1, space="PSUM") as psA2, \
                     tc.tile_pool(name="psA2s", bufs=2, space="PSUM") as psA2s:
                    for tau in range(TAU):
                        psf = psA2.tile([CC, 1536], F32, tag="ps_f")
                        psg = psA2.tile([CC, 1536], F32, tag="ps_g")
                        for vo, vl in VCH:
                            for k in range(K):
                                nc.tensor.matmul(psf[:, vo:vo + vl], wf_t[:, k, :],
                                                 H0[:, tau + k, vo:vo + vl],
                                                 start=(k == 0), stop=(k == K - 1))
                            for k in range(K):
                                nc.tensor.matmul(psg[:, vo:vo + vl], wg_t[:, k, :],
                                                 H0[:, tau + k, vo:vo + vl],
                                                 start=(k == 0), stop=(k == K - 1))
                        fs = pstag.tile([CC, NP], BF16, tag="fs")
                        gs = pstag.tile([CC, NP], BF16, tag="gs")
                        nc.scalar.activation(fs[:], psf[:, 0:NP], AF.Tanh,
                                             bias=bfv_t[:], scale=1.0)
                        nc.scalar.activation(gs[:], psg[:, 0:NP], AF.Sigmoid,
                                             bias=bgv_t[:], scale=1.0)
                        nc.vector.tensor_tensor(hcm[0:CC, tau, :], fs[:], gs[:],
                                                op=ALU.mult)

                    # residual (+ start/mixprop biases) into macc, real nodes
                    nc.vector.tensor_scalar_add(macc[:, :, 0:N],
                                                H0[:, 2:TLOC, 0:N], brv[:])
                    if debug_taps:
                        nc.gpsimd.dma_start(taps["d_H0"][:], H0[:])

                    # skip1 conv partial (contract c,tau over local range)
                    for vo, vl in VCH:
                        psum = psA2s.tile([128, 512], F32, tag="ps_s1")
                        for tau in range(TAU):
                            nc.tensor.matmul(psum[:, 0:vl], w1T_t[:, tau, :],
                                             hcm[0:CC, tau, vo:vo + vl],
                                             start=(tau == 0), stop=(tau == TAU - 1))
                        nc.vector.tensor_tensor(skip_acc[:, vo:vo + vl],
                                                skip_acc[:, vo:vo + vl],
                                                psum[:, 0:vl], op=ALU.add)

            # early pairwise AllGather of skip01 partials (overlaps mixprop)
            cc1_in = dram.tile([128, NP], F32)
            cc1_out = dram.tile([256, NP], F32)
            nc.gpsimd.dma_start(cc1_in[:], skip_acc[:])
            nc.gpsimd.collective_compute(
                "AllGather", ALU.bypass,
                ins=[cc1_in.opt()], outs=[cc1_out.opt()],
                replica_groups=[[0, 1], [2, 3], [4, 5], [6, 7]])

            # AG1 result: combine halves in place via accumulate-DMA
            # (skip_acc's own value was already copied into cc1_in)
            nc.gpsimd.dma_start(skip_acc[:], cc1_out[0:128, :])
            nc.gpsimd.dma_start(skip_acc[:], cc1_out[128:256, :],
                                accum_op=ALU.add)

            # ================= mixprop =================
            with tc.tile_pool(name="mx", bufs=1) as mx, \
                 tc.tile_pool(name="mxT", bufs=2) as mxT, \
                 tc.tile_pool(name="psM", bufs=3, space="PSUM") as ps, \
                 tc.tile_pool(name="psM2", bufs=2, space="PSUM") as ps2:
                for mp in range(2):
                    g = g_first if mp == 0 else pp.tile([128, NV, NP], BF16,
                                                        tag="G", name="g_slot2")
                    nc.gpsimd.dma_start(g[:], (g1 if mp == 0 else g2)[:])
                    wmp = wmp1_t if mp == 0 else wmp2_t
                    for q in range(NQ):
                        # node-major x^T for this row chunk, layout [vl, tau, vb, c]
                        # (xbar j-major fold: transposing hcm[:,t,:] [128c,1280v]
                        #  gives out[p=vl, j=vb, f=c])
                        xT = mxT.tile([128, 4, NV, 128], BF16, tag="xT")
                        for ti in range(4):
                            nc.sync.dma_start_transpose(xT[:, ti, :, :],
                                                        hcm[:, 4 * q + ti, :])
                        if debug_taps and mp == 0 and q == 0:
                            nc.gpsimd.dma_start(taps["d_xT"][:], xT[:])
                        hk_cm = []
                        cur = xT
                        for k in range(3):
                            nxt = mx.tile([128, 4, NV, 128], BF16,
                                          tag=f"hT{k % 2}", name=f"hT{k % 2}")
                            for vb in range(NV):
                                psum = ps.tile([128, 512], F32, tag="ps_hop")
                                for wb in range(NV):
                                    nc.tensor.matmul(
                                        psum[:],
                                        g[:, wb, 128 * vb:128 * (vb + 1)],
                                        cur[:, :, wb, :],
                                        start=(wb == 0), stop=(wb == NV - 1))
                                nc.vector.scalar_tensor_tensor(
                                    nxt[:, :, vb, :], xT[:, :, vb, :], 0.5,
                                    psum[:], ALU.mult, ALU.add)
                            # back to channel-major [c, tau, vb, vl]
                            hk = mx.tile([128, 4, NV, 128], BF16,
                                         tag=f"hkcm{k}", name=f"hkcm{k}")
                            for ti in range(4):
                                nc.sync.dma_start_transpose(hk[:, ti, :, :],
                                                            nxt[:, ti, :, :])
                            if debug_taps and mp == 0 and q == 0:
                                if k == 0:
                                    nc.gpsimd.dma_start(taps["d_h1"][:], nxt[:])
                                    nc.gpsimd.dma_start(taps["d_h1cm"][:], hk[:])
                                elif k == 1:
                                    nc.gpsimd.dma_start(taps["d_h2"][:], nxt[:])
                                else:
                                    nc.gpsimd.dma_start(taps["d_h3"][:], nxt[:])
                                    nc.gpsimd.dma_start(taps["d_h3cm"][:], hk[:])
                            hk_cm.append(hk)
                            cur = nxt
                        # conv1x1 over [x, h1, h2, h3] -> accumulate into macc
                        for ti in range(4):
                            for ci, (vo, vl) in enumerate(VCH):
                                vb0, nvb = vo // 128, vl // 128
                                psum = ps2.tile([128, vl], F32, tag="ps_mpc")
                                nc.tensor.matmul(psum[:], wmp[:, 0, :],
                                                 hcm[:, 4 * q + ti, vo:vo + vl],
                                                 start=True, stop=False)
                                for k in range(3):
                                    nc.tensor.matmul(
                                        psum[:], wmp[:, k + 1, :],
                                        hk_cm[k][:, ti, vb0:vb0 + nvb, :],
                                        start=False, stop=(k == 2))
                                hi = min(vo + vl, N)
                                nc.vector.tensor_tensor(
                                    macc[:, 4 * q + ti, vo:hi],
                                    macc[:, 4 * q + ti, vo:hi],
                                    psum[:, 0:hi - vo], op=ALU.add)
                        if mp == 1:
                            # macc rows 4q..4q+4 final: stats + rawE now
                            for ti in range(4):
                                tau = 4 * q + ti
                                nc.vector.reduce_sum(sums_t[:, tau:tau + 1],
                                                     macc[:, tau, :],
                                                     axis=mybir.AxisListType.X)
                                scr = mxT.tile([128, NP], BF16, tag="sq_scr")
                                nc.scalar.activation(scr[:], macc[:, tau, :],
                                                     AF.Square,
                                                     accum_out=sqs_t[:, tau:tau + 1])
                            for vo, vl in VCH:
                                psum = ps2.tile([128, 512], F32, tag="ps_rEq")
                                for ti in range(4):
                                    nc.tensor.matmul(
                                        psum[:, 0:vl], wET_t[:, 4 * q + ti, :],
                                        macc[:, 4 * q + ti, vo:vo + vl],
                                        start=(ti == 0), stop=(ti == 3))
                                nc.vector.tensor_tensor(rawE_acc[:, vo:vo + vl],
                                                        rawE_acc[:, vo:vo + vl],
                                                        psum[:, 0:vl], op=ALU.add)
                    if debug_taps and mp == 0:
                        nc.gpsimd.dma_start(taps["d_maccmp1"][:], macc[:])

            if debug_taps:
                nc.gpsimd.dma_start(taps["d_hcm"][:], hcm[:])
                nc.gpsimd.dma_start(taps["d_macc"][:], macc[:])
                nc.gpsimd.dma_start(taps["d_skip"][:], skip_acc[:])
            # ================= rawE + stats + collective =================
            with tc.tile_pool(name="late", bufs=1) as pl, \
                 tc.tile_pool(name="psL", bufs=1, space="PSUM") as ps:
                if debug_taps:
                    nc.gpsimd.dma_start(taps["d_rawE"][:], rawE_acc[:])

                stats_p = pl.tile([128, 2], F32)
                msum = pl.tile([128, TAU], F32)
                nc.vector.tensor_tensor(msum[:], sums_t[:], tmask_t[:], op=ALU.mult)
                nc.vector.reduce_sum(stats_p[:, 0:1], msum[:],
                                     axis=mybir.AxisListType.X)
                nc.vector.tensor_tensor(msum[:], sqs_t[:], tmask_t[:], op=ALU.mult)
                nc.vector.reduce_sum(stats_p[:, 1:2], msum[:],
                                     axis=mybir.AxisListType.X)
                if debug_taps:
                    nc.gpsimd.dma_start(taps["d_stats"][:], stats_p[:])

                cc2_in = dram.tile([128, NP + 2], F32)
                cc2_out = dram.tile([256, NP + 2], F32)
                nc.gpsimd.dma_start(cc2_in[:, 0:NP], rawE_acc[:])
                nc.gpsimd.dma_start(cc2_in[:, NP:NP + 2], stats_p[:])
                nc.gpsimd.collective_compute(
                    "AllGather", ALU.bypass,
                    ins=[cc2_in.opt()], outs=[cc2_out.opt()],
                    replica_groups=[[0, 1], [2, 3], [4, 5], [6, 7]])
                rawE_c = pl.tile([128, NP + 2], F32)
                nc.gpsimd.dma_start(rawE_c[:], cc2_out[0:128, :])
                nc.gpsimd.dma_start(rawE_c[:], cc2_out[128:256, :],
                                    accum_op=ALU.add)

                # layernorm scalars (same value on every partition)
                st_r = pl.tile([128, 2], F32)
                nc.gpsimd.partition_all_reduce(st_r[:], rawE_c[:, NP:NP + 2],
                                               channels=128,
                                               reduce_op=bass_isa.ReduceOp.add)
                mv = pl.tile([128, 1], F32)
                msqv = pl.tile([128, 1], F32)
                varv = pl.tile([128, 1], F32)
                m2v = pl.tile([128, 1], F32)
                svv = pl.tile([128, 1], F32)
                rv = pl.tile([128, 1], F32)
                rmv = pl.tile([128, 1], F32)
                bias_c = pl.tile([128, 1], F32)
                nc.vector.tensor_scalar_mul(mv[:], st_r[:, 0:1], 1.0 / CNT)
                nc.vector.tensor_scalar_mul(msqv[:], st_r[:, 1:2], 1.0 / CNT)
                nc.vector.tensor_tensor(m2v[:], mv[:], mv[:], op=ALU.mult)
                nc.vector.tensor_scalar(varv[:], msqv[:], m2v[:], EPS,
                                        op0=ALU.subtract, op1=ALU.add)
                nc.scalar.sqrt(svv[:], varv[:])
                nc.vector.reciprocal(rv[:], svv[:])
                nc.vector.tensor_scalar(rmv[:], rv[:], mv[:], -1.0,
                                        op0=ALU.mult, op1=ALU.mult)
                # bias_c = b01 - r*m*wEsum
                nc.vector.scalar_tensor_tensor(bias_c[:], wEs_t[:], rmv[:],
                                               b01_t[:], ALU.mult, ALU.add)
                # skip_pre = skip01 + r*rawE ; relu with bias
                skip_pre = pl.tile([128, NP], F32)
                nc.vector.scalar_tensor_tensor(skip_pre[:], rawE_c[:, 0:NP],
                                               rv[:], skip_acc[:],
                                               ALU.mult, ALU.add)
                rsk = pl.tile([128, NP], BF16)
                nc.vector.tensor_scalar(rsk[:], skip_pre[:], bias_c[:], 0.0,
                                        op0=ALU.add, op1=ALU.max)

                # end convs + head (wide psum, one ACT per stage)
                o1 = pl.tile([128, NP], BF16)
                o2 = pl.tile([OUT, NP], BF16)
                y_sb = pl.tile([1, NP], F32)
                ps1 = ps.tile([128, 1536], F32, tag="ps_e1")
                for vo, vl in VCH:
                    nc.tensor.matmul(ps1[:, vo:vo + vl], we1_t[:],
                                     rsk[:, vo:vo + vl], start=True, stop=True)
                nc.vector.tensor_scalar(o1[:], ps1[:, 0:NP], be1_t[:], 0.0,
                                        op0=ALU.add, op1=ALU.max)
                ps2 = ps.tile([OUT, 1536], F32, tag="ps_e2")
                for vo, vl in VCH:
                    nc.tensor.matmul(ps2[:, vo:vo + vl], we2_t[:],
                                     o1[:, vo:vo + vl], start=True, stop=True)
                nc.vector.tensor_scalar_add(o2[:], ps2[:, 0:NP], be2_t[:])
                psh = ps.tile([1, 1536], F32, tag="ps_e1", name="psh")
                for vo, vl in VCH:
                    nc.tensor.matmul(psh[:, vo:vo + vl], whT_t[:],
                                     o2[:, vo:vo + vl], start=True, stop=True)
                nc.scalar.activation(y_sb[:], psh[:, 0:NP], AF.Sigmoid,
                                     bias=bh_t[:], scale=1.0)
                nc.gpsimd.dma_start(y[:], y_sb[:])

    nc.compile()
    return nc


def _norm_adj_T_half(a):
    """0.5 * norm_adj(a).T zero-padded to [NP, NP], bf16."""
    an = a + np.eye(N, dtype=np.float32)
    an = an / an.sum(axis=1, keepdims=True)
    g = 0.5 * an.T
    gp = np.zeros((NP, NP), dtype=np.float32)
    gp[:N, :N] = g
    return gp.reshape(NV, 128, NP).transpose(1, 0, 2).astype(bf16)


def _prep_inputs(inputs):
    x = np.asarray(inputs["x"], np.float32)
    adj = np.asarray(inputs["adj"], np.float32)
    w_start = np.asarray(inputs["w_start"], np.float32)
    b_start = np.asarray(inputs["b_start"], np.float32)
    w_filt = np.asarray(inputs["w_filt"], np.float32)[:, :, 0, :]
    b_filt = np.asarray(inputs["b_filt"], np.float32)
    w_gate = np.asarray(inputs["w_gate"], np.float32)[:, :, 0, :]
    b_gate = np.asarray(inputs["b_gate"], np.float32)
    w_skip0 = np.asarray(inputs["w_skip0"], np.float32)[:, :, 0, :]
    b_skip0 = np.asarray(inputs["b_skip0"], np.float32)
    w_skip1 = np.asarray(inputs["w_skip1"], np.float32)[:, :, 0, :]
    b_skip1 = np.asarray(inputs["b_skip1"], np.float32)
    w_mp1 = np.asarray(inputs["w_mp1"], np.float32)
    b_mp1 = np.asarray(inputs["b_mp1"], np.float32)
    w_mp2 = np.asarray(inputs["w_mp2"], np.float32)
    b_mp2 = np.asarray(inputs["b_mp2"], np.float32)
    w_skipE = np.asarray(inputs["w_skipE"], np.float32)[:, :, 0, :]
    b_skipE = np.asarray(inputs["b_skipE"], np.float32)
    w_end1 = np.asarray(inputs["w_end1"], np.float32)
    b_end1 = np.asarray(inputs["b_end1"], np.float32)
    w_end2 = np.asarray(inputs["w_end2"], np.float32)
    b_end2 = np.asarray(inputs["b_end2"], np.float32)
    w_head = np.asarray(inputs["w_head"], np.float32)
    b_head = np.asarray(inputs["b_head"], np.float32)

    g1 = _norm_adj_T_half(adj)
    g2 = _norm_adj_T_half(adj.T)

    # shared (core-independent) tensors
    wsT = w_start.T  # [129, 128]
    shared = {
        "g1": g1, "g2": g2,
        "wsT_hi": wsT[:128].astype(bf16),
        "wsT_lo": wsT[128:129].astype(bf16),
        "wfT": w_filt.transpose(1, 2, 0).astype(bf16),
        "wgT": w_gate.transpose(1, 2, 0).astype(bf16),
        "bf_v": (b_filt + w_filt.sum(2) @ b_start).reshape(CC, 1).astype(np.float32),
        "bg_v": (b_gate + w_gate.sum(2) @ b_start).reshape(CC, 1).astype(np.float32),
        "b_resid_v": (b_start + b_mp1 + b_mp2).reshape(128, 1).astype(np.float32),
        "wEsum_v": w_skipE.sum((1, 2)).reshape(128, 1).astype(np.float32),
        "b01_v": (b_skip0 + b_skip1 + b_skipE).reshape(128, 1).astype(np.float32),
        "we1T": w_end1.T.astype(bf16),
        "be1_v": b_end1.reshape(128, 1).astype(np.float32),
        "we2T": w_end2.T.astype(bf16),
        "be2_v": b_end2.reshape(OUT, 1).astype(np.float32),
        "whT": w_head.T.astype(bf16),
        "bh_v": b_head.reshape(1, 1).astype(np.float32),
    }
    # w_mp as [c(128 pad), k, o]
    for nm, w in (("wmp1T", w_mp1), ("wmp2T", w_mp2)):
        arr = np.zeros((128, 4, 128), np.float32)
        for k in range(4):
            arr[:CC, k, :] = w[:, k * CC:(k + 1) * CC].T
        shared[nm] = arr.astype(bf16)

    in_maps = []
    for core in range(8):
        b, th = core // 2, core % 2
        t_lo = 0 if th == 0 else TAU
        # x slice [129, 1280, TLOC] zero-padded in nodes and t
        xp = np.zeros((C_IN, TLOC, NP), np.float32)
        t_hi = min(t_lo + TLOC, T)
        xp[:, 0:t_hi - t_lo, :N] = x[b, :, :, t_lo:t_hi].transpose(0, 2, 1)
        # skip0 weight slots aligned to local t: core owns t range
        w0T = np.zeros((C_IN, TLOC, 128), np.float32)
        own_lo, own_hi = (0, 13) if th == 0 else (13, T)
        for tp_ in range(TLOC):
            tg = t_lo + tp_
            if own_lo <= tg < own_hi:
                w0T[:, tp_, :] = w_skip0[:, :, tg].T
        # skip1 / skipE weight slots aligned to local tau
        w1Ta = np.zeros((CC, TAU, 128), np.float32)
        wETa = np.zeros((128, TAU, 128), np.float32)
        for tau in range(TAU):
            tg = t_lo + tau
            if tg < T1:
                w1Ta[:, tau, :] = w_skip1[:, :, tg].T
                wETa[:, tau, :] = w_skipE[:, :, tg].T
        tm = np.ones((128, TAU), np.float32)
        if th == 1:
            tm[:, T1 - TAU:] = 0.0  # tau slots beyond T1 are padding
        m = dict(shared)
        m["x_hi"] = xp[:128].astype(bf16)
        m["x_lo"] = xp[128].astype(bf16)
        m["w0T_hi"] = w0T[:128].astype(bf16)
        m["w0T_lo"] = w0T[128:129].astype(bf16)
        m["w1T"] = w1Ta.astype(bf16)
        m["wET"] = wETa.astype(bf16)
        m["tmask"] = tm
        in_maps.append(m)
    return in_maps


def kernel(**inputs):
    if "nc" not in _CACHE:
        _CACHE["nc"] = _build_program()
    nc = _CACHE["nc"]
    in_maps = _prep_inputs(inputs)
    res = bass_utils.run_bass_kernel_spmd(nc, in_maps, core_ids=list(range(8)))
    out = np.empty((B, N), np.float32)
    for b in range(B):
        out[b] = res.results[2 * b]["y"][0, :N]
    return out
